# revision 32
# baseline (speedup 1.0000x reference)
"""Trainium2 Bass kernel for DecoderAttention (b=2, n=2048, m=1024, d=1024, h=16).

Sharding: 8 cores = 2 (batch) x 4 (head groups of 4 heads).  Each core:
  - projects q/k/v for its 4 heads from x|context (pre-transposed on host),
  - runs causal flash attention in scores-transposed layout [kj, qi]
    (softmax without max subtraction -- scores are bounded; causally masked
    entries multiply to exactly 0 after exp, matching exp(-50000)),
  - computes its partial out-projection  attn_out_g @ Wo[rows_g]  [2048, 1024].
Host sums the 4 head-group partials per batch (the "all-reduce") and adds bo.

All matmuls run in bf16 with f32 PSUM accumulation (validated ~0.5% rel err).

v5 schedule notes (HW-profile driven; v2 was 302us, PE 75% busy):
  - Every input is repacked on the HOST into its exact SBUF layout, so each
    DMA is one descriptor per partition (4-8KB contiguous reads).  The v2-v4
    weight/xk slices generated 256B-1KB descriptors and startup DMAs ran at
    ~180GB/s aggregate, gating the first matmul to ~24us.
  - Startup DMAs ride the two HARDWARE DGE queues in deadline order (sync:
    xk chunk-0, mt0 q/k weights, ctx halves; scalar: mask, Wv, mt1
    weights); gpsimd issues no DMAs so its mask multiplies never queue.
  - AV matmuls are deferred THREE rounds behind their scores: ps_s bufs=2
    forces exp(r-2) complete before QK(r) starts, and the extra round
    tolerates the occasional ~4us GPSIMD mask-multiply hiccup that stalled
    the 2-round schedule at segment boundaries.  The two heads' diagonal
    mask multiplies split DVE/GPSIMD so neither queue backs up.
  - Diagonal score tiles for chunks >= 1 skip the causally-dead leading
    query columns in both the QK matmul and the exp (3D strided AP over the
    two heads); the full-width mask multiply re-zeroes the stale region.
    Chunk 0 stays full-width so every pt ring slot holds finite data before
    its first sliced reuse (no startup ring memsets).
  - The per-pair normalize is split three ways and placed by hard deadline
    in the next segment: accumulator eviction at round 0 (frees the AV psum
    slots), the [33,512] DVE reciprocal + recb at round 2 (behind that
    round's mask mul in the DVE FIFO), and the PE broadcast + aT scale at
    round 5 (after the reciprocal has drained).  Outproj fillers spread
    strictly later.  The final chunk normalizes per 128-col quarter,
    pipelined with its outproj units, and its odd out-DMAs issue on gpsimd
    so descriptor-gen doesn't serialize the tail.
"""

import os

# The neuron/axon jax backend must be discoverable for the PJRT execution
# path; a JAX_PLATFORMS=cpu pin (used when running the jax reference) would
# hide the trn2 devices from this process.
if os.environ.get("JAX_PLATFORMS", "").strip().lower() == "cpu":
    del os.environ["JAX_PLATFORMS"]

from contextlib import ExitStack

import ml_dtypes
import numpy as np

import concourse.bass as bass
import concourse.tile as tile
from concourse import bacc, mybir
from concourse.bass_utils import run_bass_kernel_spmd

B, N, M, D = 2, 2048, 1024, 1024
H, DH = 16, 64
NM = N + M          # 3072 keys (self + context)
GROUPS = 4          # head groups; 4 heads = 256 cols per group
GC = 256            # columns per head group
NCORES = 8
SCALE = DH ** -0.5
P = 128
KT = D // P         # 8 contraction tiles over d
QCH = 512           # query-chunk width
NQC = N // QCH      # 4 query chunks
NBLK = NM // QCH    # 6 column blocks of xk
NKJ = NM // P       # 24 key tiles
NSELF = N // P      # 16 self key tiles
PTS = 8             # pt ring slots
AVD = 5             # AV deferral depth (rounds); AVD+1 pt slots live
FP32 = mybir.dt.float32
BF16 = mybir.dt.bfloat16
BF16NP = ml_dtypes.bfloat16


def _active_kj(c):
    """Key tiles with any unmasked entry for query chunk c (512 queries).

    Chunk 0 runs its (diagonal) self tiles first since the cross columns
    arrive later over DMA.  Later chunks run cross tiles first -- so the
    chunk's own self-k/v projections can be produced as same-segment
    fillers -- with the 4 masked diagonal tiles interleaved (positions
    2,4,6,8) so their mask multiplies spread across the segment instead of
    serializing at its end."""
    if c == 0:
        return list(range(0, 4)) + list(range(NSELF, NKJ))
    d = list(range(4 * c, 4 * c + 4))
    x = list(range(NSELF, NKJ))
    return ([x[0], x[1], d[0], x[2], d[1], x[3], d[2], x[4], d[3]]
            + x[5:] + list(range(0, 4 * c)))


def _build_module(biased: bool):
    nc = bacc.Bacc(
        "TRN2",
        target_bir_lowering=False,
        debug=False,
        enable_asserts=False,
        num_devices=NCORES,
    )
    # all inputs pre-packed on host to SBUF layout: one contiguous
    # descriptor per partition per DMA
    xkvT_d = nc.dram_tensor(
        "xkvT", [P, NBLK * KT * QCH], BF16, kind="ExternalInput").ap()
    wq_d = nc.dram_tensor("wq", [P, 2 * KT * P], BF16, kind="ExternalInput").ap()
    wk_d = nc.dram_tensor("wk", [P, 2 * KT * P], BF16, kind="ExternalInput").ap()
    wv_d = nc.dram_tensor("wv", [P, KT * GC], BF16, kind="ExternalInput").ap()
    wo_d = nc.dram_tensor("wo", [P, 2 * D], BF16, kind="ExternalInput").ap()
    msk_d = nc.dram_tensor("msk", [P, 4 * QCH], BF16, kind="ExternalInput").ap()
    eye_d = nc.dram_tensor("eye", [P, P], BF16, kind="ExternalInput").ap()
    if biased:
        bq_d = nc.dram_tensor("bq", [1, GC], BF16, kind="ExternalInput").ap()
        bk_d = nc.dram_tensor("bk", [1, GC], BF16, kind="ExternalInput").ap()
        bv_d = nc.dram_tensor("bv", [1, GC], BF16, kind="ExternalInput").ap()
    out_d = nc.dram_tensor("out", [N, D], BF16, kind="ExternalOutput").ap()

    with tile.TileContext(nc) as tc, ExitStack() as ctx:
        const = ctx.enter_context(tc.tile_pool(name="const", bufs=1))
        bcp = ctx.enter_context(tc.tile_pool(name="bcp", bufs=3))
        osbp = ctx.enter_context(tc.tile_pool(name="osbp", bufs=3))
        # PSUM budget: 8 banks = proj/psb(2) + scores(2x2) + av(2)
        ps_main = ctx.enter_context(tc.tile_pool(name="ps_main", bufs=2, space="PSUM"))
        ps_s = ctx.enter_context(tc.tile_pool(name="ps_s", bufs=2, space="PSUM"))
        ps_av = ctx.enter_context(tc.tile_pool(name="ps_av", bufs=2, space="PSUM"))

        # ---- persistent SBUF tensors (column-concatenated k-tiles) ----
        xk = const.tile([P, KT * NM], BF16)          # xkvT: 8 tiles of [128, 3072]
        wqs = const.tile([P, 2 * KT * P], BF16)      # mt-major, then kt
        wks = const.tile([P, 2 * KT * P], BF16)
        wvs = const.tile([P, KT * GC], BF16)         # kt-major
        wos = const.tile([P, 2 * D], BF16)           # Wo rows: 2 tiles of [128, 1024]
        mks = const.tile([P, 4 * QCH], BF16)         # 4 diagonal mask tiles
        qT = const.tile([P, 2 * N], BF16)            # [head-pair cols, qi]
        kT = const.tile([P, 2 * NM], BF16)           # [head-pair cols, kj]
        vT = const.tile([P, 2 * NM], BF16)           # [head-pair v-cols, token]
        vv = const.tile([P, NKJ * 4 * 65], BF16)     # per kj tile: 4x [v(64)|1]
        eye = const.tile([P, P], BF16)               # identity for PE transpose
        aT = const.tile([P, 2 * N], BF16)            # attn_out^T, 2 k-tiles
        ptr = const.tile([P, PTS * 2 * QCH], BF16)   # pt ring (exp'd scores)
        den_t = const.tile([33, 2 * QCH], FP32)      # per-pair den seeds @rows 0/32
        ones_l = const.tile([1, 64], BF16)
        dummy = const.tile([1, 2], FP32)
        if biased:
            bq_s = const.tile([1, GC], BF16)
            bk_s = const.tile([1, GC], BF16)
            bv_s = const.tile([1, GC], BF16)
            ones_row = const.tile([1, QCH], BF16)
            ones_col = const.tile([1, P], BF16)

        # ---- ACT table preload: a dummy exp during the DMA window ----
        nc.vector.memset(dummy[:], 1.0)
        nc.scalar.activation(
            dummy[:, 0:1], dummy[:, 1:2], mybir.ActivationFunctionType.Exp
        )

        xk3 = xk.rearrange("p (kt nm) -> p kt nm", kt=KT)
        xp4 = xkvT_d.rearrange("p (b kt q) -> p b kt q", b=NBLK, kt=KT)
        wq2 = wq_d.rearrange("p (mt r) -> p mt r", mt=2)
        wk2 = wk_d.rearrange("p (mt r) -> p mt r", mt=2)
        wqs2 = wqs.rearrange("p (mt r) -> p mt r", mt=2)
        wks2 = wks.rearrange("p (mt r) -> p mt r", mt=2)

        def dma_blk(eng, blk):  # xk cols [blk*512, +512), all kt tiles
            eng.dma_start(xk3[:, :, blk * QCH:(blk + 1) * QCH], xp4[:, blk])

        # sync queue, deadline order: mt0 weights (small) lead so they never
        # queue behind the 1MB chunk-0 block, which is itself split per
        # 4-kt half so the first k-proj matmuls start ~2us earlier
        nc.sync.dma_start(wqs2[:, 0], wq2[:, 0])
        nc.sync.dma_start(wks2[:, 0], wk2[:, 0])
        nc.sync.dma_start(xk3[:, 0:4, 0:QCH], xp4[:, 0, 0:4])
        nc.sync.dma_start(xk3[:, 4:KT, 0:QCH], xp4[:, 0, 4:KT])
        dma_blk(nc.sync, 4)   # ctx first half (kT(0,4) deadline ~round 3)
        dma_blk(nc.sync, 5)
        # scalar queue: identity (first transposes), mask (round-0 mask
        # mul), Wv (round ~1), mt1 weights (pair 1)
        nc.scalar.dma_start(eye[:], eye_d[:])
        nc.scalar.dma_start(mks[:], msk_d[:])
        nc.scalar.dma_start(wvs[:], wv_d[:])
        nc.scalar.dma_start(wqs2[:, 1], wq2[:, 1])
        nc.scalar.dma_start(wks2[:, 1], wk2[:, 1])

        def dma_mid(j):
            dma_blk(nc.sync, j)

        def dma_wos():
            nc.sync.dma_start(wos[:], wo_d[:])
        nc.vector.memset(ones_l[:], 1.0)
        nc.vector.memset(den_t[:], 1.0)  # rows between head seeds stay finite
        if biased:
            nc.sync.dma_start(bq_s[:], bq_d[:])
            nc.sync.dma_start(bk_s[:], bk_d[:])
            nc.sync.dma_start(bv_s[:], bv_d[:])
            nc.vector.memset(ones_row[:], 1.0)
            nc.vector.memset(ones_col[:], 1.0)
        # ones columns interleaved into vv: col (t*260 + h*65 + 64)
        nc.gpsimd.memset(
            vv.rearrange("p (t h x) -> p t h x", t=NKJ, h=4)[:, :, :, 64:65], 1.0
        )

        # ---- emission helpers ----
        def emit_qT_group(mt, c):
            psq = ps_main.tile([P, QCH], FP32, tag="proj", name="psq")
            for kt in range(KT):
                nc.tensor.matmul(
                    psq[:],
                    lhsT=wqs[:, (mt * KT + kt) * P:(mt * KT + kt + 1) * P],
                    rhs=xk[:, kt * NM + c * QCH: kt * NM + (c + 1) * QCH],
                    start=(kt == 0),
                    stop=(kt == KT - 1) and not biased,
                )
            if biased:
                nc.tensor.matmul(
                    psq[:], lhsT=bq_s[:, mt * P:(mt + 1) * P], rhs=ones_row[:],
                    start=False, stop=True,
                )
            nc.vector.tensor_copy(
                qT[:, mt * N + c * QCH: mt * N + (c + 1) * QCH], psq[:]
            )

        def emit_kT_group(mt, c2):
            psk = ps_main.tile([P, QCH], FP32, tag="proj", name="psk")
            for kt in range(KT):
                nc.tensor.matmul(
                    psk[:],
                    lhsT=wks[:, (mt * KT + kt) * P:(mt * KT + kt + 1) * P],
                    rhs=xk[:, kt * NM + c2 * QCH: kt * NM + (c2 + 1) * QCH],
                    start=(kt == 0),
                    stop=(kt == KT - 1) and not biased,
                )
            if biased:
                nc.tensor.matmul(
                    psk[:], lhsT=bk_s[:, mt * P:(mt + 1) * P], rhs=ones_row[:],
                    start=False, stop=True,
                )
            nc.vector.tensor_copy(
                kT[:, mt * NM + c2 * QCH: mt * NM + (c2 + 1) * QCH], psk[:]
            )

        def emit_vT_group(mt, c2):
            # v projected in kT orientation (512-free matmuls fully hide
            # the LDWEIGHTS; the old token-stationary form ran 256-free at
            # ~2.5x the cycle cost), then PE-transposed per kj tile below
            psv = ps_main.tile([P, QCH], FP32, tag="proj", name="psvT")
            for kt in range(KT):
                nc.tensor.matmul(
                    psv[:],
                    lhsT=wvs[:, kt * GC + mt * P: kt * GC + (mt + 1) * P],
                    rhs=xk[:, kt * NM + c2 * QCH: kt * NM + (c2 + 1) * QCH],
                    start=(kt == 0),
                    stop=(kt == KT - 1) and not biased,
                )
            if biased:
                nc.tensor.matmul(
                    psv[:], lhsT=bv_s[:, mt * P:(mt + 1) * P], rhs=ones_row[:],
                    start=False, stop=True,
                )
            nc.vector.tensor_copy(
                vT[:, mt * NM + c2 * QCH: mt * NM + (c2 + 1) * QCH], psv[:]
            )

        def emit_vtr(t, mt):
            pst = ps_main.tile([P, P], BF16, tag="proj", name="pst")
            nc.tensor.transpose(
                pst[:], vT[:, mt * NM + t * P: mt * NM + (t + 1) * P], eye[:]
            )
            nc.vector.tensor_copy(
                vv[:, t * 260 + 2 * mt * 65: t * 260 + 2 * mt * 65 + 130]
                .rearrange("p (h x) -> p h x", h=2)[:, :, 0:64],
                pst.rearrange("p (h x) -> p h x", h=2),
            )

        def emit_outproj_unit(c, it, nh):
            pso = ps_main.tile([P, QCH], FP32, tag="proj", name="pso")
            for kt in range(2):
                nc.tensor.matmul(
                    pso[:],
                    lhsT=aT[:, kt * N + it * P: kt * N + (it + 1) * P],
                    rhs=wos[:, kt * D + nh * QCH: kt * D + (nh + 1) * QCH],
                    start=(kt == 0),
                    stop=(kt == 1),
                )
            osb = osbp.tile([P, QCH], BF16, tag="osb", name="osb")
            nc.vector.tensor_copy(osb[:], pso[:])
            # the last chunk's 8 units drain at the very end: split their
            # DMA issues across two queues so descriptor-gen (~1us apiece)
            # doesn't serialize the tail
            eng = nc.gpsimd if (c == NQC - 1 and nh == 1) else nc.sync
            eng.dma_start(
                out_d[it * P:(it + 1) * P, nh * QCH:(nh + 1) * QCH], osb[:]
            )

        rot = [0]  # pt ring rotation

        def emit_attention_segment(c, pair, fillers, chunk_ctx, hard=()):
            """One (chunk, head-pair) flash segment with interleaved filler.

            `hard` fillers are (deadline_round, fn): fn EMITS data consumed
            by this segment's own later rounds, so it must be emitted (and
            thus dep-tracked as the writer) before the consuming round --
            an after-the-reader write becomes a WAR hazard and the reader
            deterministically sees uninitialized SBUF.  `fillers` are
            order-free (consumed only by later segments) and are spread
            evenly for scheduler priority."""
            kjs = _active_kj(c)
            last = len(kjs) - 1
            nfill = len(fillers)
            fdone = 0
            hard = list(hard)
            ps_acc = [None, None]
            pending = []  # up to AVD rounds of exp'd tiles not yet fed to AV

            def do_av(pts, i):
                # NOTE: all members of this accumulation group must keep the
                # SAME output AP -- column-sliced members corrupt the bank's
                # has_written state on real hardware (sim doesn't model it).
                t = kjs[i]
                for hh in range(2):
                    h = pair * 2 + hh
                    nc.tensor.matmul(
                        ps_acc[hh][:],
                        lhsT=vv[:, t * 260 + h * 65: t * 260 + (h + 1) * 65],
                        rhs=pts[hh],
                        start=(i == 0),
                        stop=(i == last),
                    )

            for i, t in enumerate(kjs):
                diag = 4 * c <= t < 4 * c + 4
                dt = t - 4 * c if diag else 0
                # chunks >= 1 skip the causally-dead leading q columns of
                # diagonal tiles; chunk 0 stays full width so the pt ring
                # slots hold finite data before their first sliced reuse
                sl = 128 * dt if c > 0 else 0
                pss = ps_s.tile([P, 2 * QCH], FP32, tag="s", name="pss")
                for hh in range(2):
                    lo, hi = hh * 64, hh * 64 + 64
                    nc.tensor.matmul(
                        pss[:, hh * QCH + sl:(hh + 1) * QCH],
                        lhsT=kT[lo:hi, pair * NM + t * P: pair * NM + (t + 1) * P],
                        rhs=qT[lo:hi,
                               pair * N + c * QCH + sl: pair * N + (c + 1) * QCH],
                        start=True,
                        stop=True,
                    )
                slot = rot[0] % PTS
                rot[0] += 1
                pt = ptr[:, slot * 2 * QCH:(slot + 1) * 2 * QCH]
                if sl:
                    nc.scalar.activation(
                        pt.rearrange("p (h q) -> p h q", h=2)[:, :, sl:],
                        pss.rearrange("p (h q) -> p h q", h=2)[:, :, sl:],
                        mybir.ActivationFunctionType.Exp,
                    )
                else:
                    nc.scalar.activation(
                        pt, pss[:], mybir.ActivationFunctionType.Exp
                    )
                if diag:  # causal mask; full width also re-zeroes stale cols
                    # one head per engine: a single queue serializes on the
                    # exp arrivals (each mask waits its exp at the FIFO
                    # head) and drifts ~1us per diagonal round; the 5-round
                    # AV deferral absorbs the residual latency of both
                    for hh in range(2):
                        eng = nc.vector if hh == 0 else nc.gpsimd
                        eng.tensor_mul(
                            pt[:, hh * QCH:(hh + 1) * QCH],
                            pt[:, hh * QCH:(hh + 1) * QCH],
                            mks[:, dt * QCH:(dt + 1) * QCH],
                        )
                if i == 0:
                    ps_acc[0] = ps_av.tile([65, QCH], FP32, tag="av", name="av0")
                    ps_acc[1] = ps_av.tile([65, QCH], FP32, tag="av", name="av1")
                if len(pending) >= AVD:
                    do_av(*pending.pop(0))
                pending.append(
                    ([pt[:, hh * QCH:(hh + 1) * QCH] for hh in range(2)], i)
                )
                # deadline fillers first, then spread the order-free ones
                # (reserving a few for the end-of-segment exp drain)
                while hard and hard[0][0] <= i:
                    hard.pop(0)[1]()
                want = (i + 1) * nfill // (len(kjs) + AVD)
                while fdone < want:
                    fillers[fdone]()
                    fdone += 1
            for _, f in hard:
                f()
            while fdone < nfill:
                fillers[fdone]()
                fdone += 1
            for p_ in pending:
                do_av(*p_)

            # normalize is split three ways, placed by hard deadline in the
            # next segment, so the DVE reciprocal never delays that
            # segment's round-2 mask multiply and the PE broadcast never
            # queues ahead of the unfinished reciprocal:
            #   norm_a (round 0) -- DVE eviction of the accumulators (frees
            #     the AV psum slots for the next segment's round-AVD AV).
            #   norm_r (round 2) -- per-pair [33,512] reciprocal (heads at
            #     partitions 0/32) + recb evictions.
            #   norm_b (round 5) -- PE broadcast + aT scale.  norm_b(qtr=j)
            #     runs one 128-col quarter with its own reciprocal (skip
            #     norm_r) so the final chunk's tail pipelines recip ->
            #     broadcast -> outproj per quarter.
            # (reciprocal_approx_fast is numerically broken on HW via this
            # runtime -- keep the stock iterative reciprocal.)
            nstate = {}

            def norm_a():
                den = den_t[:, pair * QCH:(pair + 1) * QCH]
                nstate["den"] = den
                for hh in range(2):
                    h = pair * 2 + hh
                    nc.vector.tensor_copy(
                        den[32 * hh:32 * hh + 1, :], ps_acc[hh][64:65, :]
                    )
                    unrm = bcp.tile(
                        [64, QCH], BF16, tag="unrm", bufs=5, name="unrm"
                    )
                    # (GPSIMD cannot read PSUM -- evictions must stay DVE)
                    nc.vector.tensor_copy(unrm[:], ps_acc[hh][0:64, :])
                    chunk_ctx[("unrm", h)] = unrm

            def norm_r():
                rec2 = bcp.tile([33, QCH], FP32, tag="rec2", bufs=2,
                                name="rec2")
                # rows between the 0/32 seeds are junk; never read
                nc.vector.reciprocal(rec2[:], nstate["den"][:])
                recbs = []
                for hh in range(2):
                    recb = bcp.tile([1, QCH], BF16, tag="recb", bufs=3,
                                    name="recb")
                    nc.vector.tensor_copy(recb[:], rec2[32 * hh:32 * hh + 1, :])
                    recbs.append(recb)
                nstate["recbs"] = recbs

            def norm_b(qtr=None):
                if qtr is None:
                    q0, qw = 0, QCH
                    recbs = nstate["recbs"]
                else:
                    q0, qw = qtr * P, P
                    rec2 = bcp.tile([33, P], FP32, tag="rec2q", bufs=2,
                                    name="rec2q")
                    nc.vector.reciprocal(rec2[:], nstate["den"][:, q0:q0 + qw])
                    recbs = []
                    for hh in range(2):
                        recb = bcp.tile([1, P], BF16, tag="recbq", bufs=3,
                                        name="recbq")
                        nc.vector.tensor_copy(
                            recb[:], rec2[32 * hh:32 * hh + 1, :]
                        )
                        recbs.append(recb)
                for hh in range(2):
                    h = pair * 2 + hh
                    # TensorE broadcast of the reciprocal row: a GPSIMD
                    # partition_broadcast would be cheaper on paper, but
                    # custom GPSIMD ops live in a different Q7 library than
                    # tensor_tensor and every call forces a ~6us library
                    # swap that stalls the mask-multiply FIFO
                    psb = ps_main.tile([64, qw], FP32, tag="proj", name="psb")
                    nc.tensor.matmul(
                        psb[:], lhsT=ones_l[:], rhs=recbs[hh][:],
                        start=True, stop=True,
                    )
                    kt2 = h // 2
                    lo = (h % 2) * 64
                    nc.vector.tensor_mul(
                        aT[lo:lo + 64,
                           kt2 * N + c * QCH + q0:
                           kt2 * N + c * QCH + q0 + qw],
                        chunk_ctx[("unrm", h)][:, q0:q0 + qw],
                        psb[:],
                    )

            return norm_a, norm_r, norm_b

        # ---- startup projections: minimum prefix for chunk-0 pair-0.
        # Everything else is emitted as segment filler so its scheduler
        # priority sits BELOW the score rounds it must not delay. ----
        emit_kT_group(0, 0)
        emit_qT_group(0, 0)

        # ---- main stream: attention segments with interleaved filler ----
        def outproj_fillers(c):
            f = []
            for it in range(4 * c, 4 * c + 4):
                for nh in range(2):
                    f.append(lambda it=it, nh=nh: emit_outproj_unit(c, it, nh))
            return f

        qg = lambda mt, c: (lambda: emit_qT_group(mt, c))
        kg = lambda mt, c2: (lambda: emit_kT_group(mt, c2))
        vtg = lambda mt, c2: (lambda: emit_vT_group(mt, c2))
        vtr = lambda t, mt: (lambda: emit_vtr(t, mt))
        spacer = lambda: None

        # segment (0,0): this pair's v transposes (consumed by the deferred
        # AV from round i+AVD), cross-key/value projections (needed from
        # round 4, in kj order) and pair-1's q/k.  Later chunks run cross-
        # first, so each chunk's own self-k/v projections ride as earlier
        # fillers and only their transposes keep hard deadlines.
        # chunk-0 kjs = [0..3, 16..23]: cross tile 16+j consumed at round
        # 4+j (QK) / 4+j+AVD (AV); its kT/vT group and transpose must be
        # emitted strictly earlier.
        hard00 = (
            [(0, vtg(0, 0)), (0, vtr(0, 0)), (1, vtr(1, 0)),
             (2, vtr(2, 0)), (2, vtr(3, 0)),
             (3, kg(0, 4)), (3, vtg(0, 4)), (4, vtr(16, 0)),
             (5, vtr(17, 0)), (6, vtr(18, 0)), (6, kg(0, 5)),
             (7, vtg(0, 5)), (7, vtr(19, 0)), (8, vtr(20, 0)),
             (9, vtr(21, 0)), (10, vtr(22, 0)), (11, vtr(23, 0))]
        )
        cctx = {}
        n0a, n0r, n0b = emit_attention_segment(
            0, 0,
            [lambda: dma_mid(1), dma_wos, qg(1, 0), kg(1, 0),
             kg(1, 4), kg(1, 5), vtg(1, 0), vtg(1, 4), vtg(1, 5)],
            cctx, hard=hard00,
        )
        hard01 = (
            [(0, vtr(0, 1)), (1, vtr(1, 1)), (2, n0a), (2, vtr(2, 1)),
             (3, n0r), (3, vtr(3, 1)), (4, vtr(16, 1)), (5, vtr(17, 1)),
             (6, n0b), (6, vtr(18, 1)), (7, vtr(19, 1)), (8, vtr(20, 1)),
             (9, vtr(21, 1)), (10, vtr(22, 1)), (11, vtr(23, 1))]
        )
        pa, pr, pb = emit_attention_segment(
            0, 1,
            [lambda: dma_mid(2), qg(0, 1), qg(1, 1), kg(0, 1),
             vtg(0, 1), vtg(1, 1)],
            cctx, hard=hard01,
        )
        for c in range(1, NQC):
            op = outproj_fillers(c - 1)
            # chunk-c diag tiles sit at kjs positions 2,4,6,8 -> AV at
            # 7,9,11,13; their transposes ride as hard fillers.  outproj
            # fillers sit late in the spread so the chunk c-1 pair-1
            # aT-scale (norm_b at round 6 + DVE drain) completes first.
            dg = [4 * c + j for j in range(4)]
            fillA = [kg(1, c),
                     (qg(0, c + 1) if c < NQC - 1 else spacer)] + op[:3]
            hardA = [(2, pa), (3, pr), (4, vtr(dg[0], 0)), (6, pb),
                     (6, vtr(dg[1], 0)), (8, vtr(dg[2], 0)),
                     (10, vtr(dg[3], 0))]
            cctx = {}
            ca, cr, cb = emit_attention_segment(c, 0, fillA, cctx, hard=hardA)
            fillB = op[3:]
            hardB = [(2, ca), (3, cr), (4, vtr(dg[0], 1)), (6, cb),
                     (6, vtr(dg[1], 1)), (8, vtr(dg[2], 1)),
                     (10, vtr(dg[3], 1))]
            if c < NQC - 1:
                nx = c + 1
                fillB += [qg(1, nx), kg(0, nx), vtg(0, nx), vtg(1, nx)]
                if nx == 2:
                    fillB.insert(1, lambda: dma_mid(3))
            pa, pr, pb = emit_attention_segment(
                c, 1, fillB, cctx, hard=hardB
            )
        # tail: per-quarter reciprocal -> broadcast -> outproj pipeline so
        # the final units start ~1us after the last AV instead of waiting
        # the full [33,512] reciprocal chain
        pa()
        op = outproj_fillers(NQC - 1)
        for qtr in range(4):
            pb(qtr=qtr)
            op[2 * qtr]()
            op[2 * qtr + 1]()

    nc.compile()
    return nc


_CACHE: dict = {}


def _module(biased: bool):
    if biased not in _CACHE:
        _CACHE[biased] = _build_module(biased)
    return _CACHE[biased]


def _pack_kt(a):
    """[KT*P, C] -> [P, KT*C]: kt-major columns, contiguous per partition."""
    c = a.shape[1]
    return np.ascontiguousarray(
        a.reshape(KT, P, c).transpose(1, 0, 2).reshape(P, KT * c)
    )


def _pack_mt_kt(a):
    """[KT*P, 2*P] -> [P, 2*KT*P]: mt-major then kt, contiguous."""
    return np.ascontiguousarray(
        a.reshape(KT, P, 2, P).transpose(1, 2, 0, 3).reshape(P, 2 * KT * P)
    )


def _pack_xkv(xt):
    """[D, NM] -> [P, NBLK*KT*QCH]: 512-col blocks, kt-major inside."""
    return np.ascontiguousarray(
        xt.reshape(KT, P, NBLK, QCH).transpose(1, 2, 0, 3).reshape(P, -1)
    )


def _mask_tiles():
    t = np.arange(4)[:, None, None]
    p = np.arange(P)[None, :, None]
    q = np.arange(QCH)[None, None, :]
    m = (p + P * t <= q).astype(BF16NP)          # [4, P, QCH]
    return np.ascontiguousarray(m.transpose(1, 0, 2).reshape(P, 4 * QCH))


def kernel(x, context, Wq, bq, Wkv, bkv, Wo, bo, mask, context_mask):
    assert bool(np.all(mask)) and bool(np.all(context_mask)), (
        "only all-true padding masks are supported"
    )
    x = np.asarray(x, np.float32)
    context = np.asarray(context, np.float32)
    Wq, bq = np.asarray(Wq, np.float32), np.asarray(bq, np.float32)
    Wkv, bkv = np.asarray(Wkv, np.float32), np.asarray(bkv, np.float32)
    Wo, bo = np.asarray(Wo, np.float32), np.asarray(bo, np.float32)

    biased = bool(np.any(bq) or np.any(bkv))
    nc = _module(biased)

    msk = _mask_tiles()
    xkvT = [
        _pack_xkv(
            np.concatenate([x[b], context[b]], axis=0).T.astype(BF16NP)
        )
        for b in range(B)
    ]
    in_maps = []
    for core in range(NCORES):
        b, g = divmod(core, GROUPS)
        cols = slice(g * GC, (g + 1) * GC)
        im = {
            "xkvT": xkvT[b],
            "wq": _pack_mt_kt((Wq[:, cols] * SCALE).astype(BF16NP)),
            "wk": _pack_mt_kt(Wkv[:, cols].astype(BF16NP)),
            "wv": _pack_kt(Wkv[:, D + g * GC: D + (g + 1) * GC].astype(BF16NP)),
            "wo": np.ascontiguousarray(
                Wo[cols, :].reshape(2, P, D).transpose(1, 0, 2).reshape(P, 2 * D)
            ).astype(BF16NP),
            "msk": msk,
            "eye": np.eye(P, dtype=BF16NP),
        }
        if biased:
            im["bq"] = (bq[cols] * SCALE).astype(BF16NP).reshape(1, GC)
            im["bk"] = bkv[cols].astype(BF16NP).reshape(1, GC)
            im["bv"] = bkv[D + g * GC: D + (g + 1) * GC].astype(BF16NP).reshape(1, GC)
        in_maps.append(im)

    try:
        res = run_bass_kernel_spmd(nc, in_maps, core_ids=list(range(NCORES)))
    except ModuleNotFoundError:
        # BASS_TRACE set but the NTFF profiling hook isn't available in this
        # environment -- rerun with tracing hard-disabled.
        os.environ["BASS_NEVER_TRACE"] = "1"
        res = run_bass_kernel_spmd(nc, in_maps, core_ids=list(range(NCORES)))
    kernel.last_results = res
    out = np.zeros((B, N, D), np.float32)
    for core in range(NCORES):
        b = core // GROUPS
        out[b] += np.asarray(res.results[core]["out"], dtype=np.float32)
    out += bo
    return out


# revision 38
# speedup vs baseline: 1.0188x; 1.0188x over previous
"""Trainium2 Bass kernel for DecoderAttention (b=2, n=2048, m=1024, d=1024, h=16).

Sharding: 8 cores = 2 (batch) x 4 (head groups of 4 heads).  Each core:
  - projects q/k/v for its 4 heads from x|context (pre-transposed on host),
  - runs causal flash attention in scores-transposed layout [kj, qi]
    (softmax without max subtraction -- scores are bounded; causally masked
    entries multiply to exactly 0 after exp, matching exp(-50000)),
  - computes its partial out-projection  attn_out_g @ Wo[rows_g]  [2048, 1024].
Host sums the 4 head-group partials per batch (the "all-reduce") and adds bo.

All matmuls run in bf16 with f32 PSUM accumulation (validated ~0.5% rel err).

v5 schedule notes (HW-profile driven; v2 was 302us, PE 75% busy):
  - Every input is repacked on the HOST into its exact SBUF layout, so each
    DMA is one descriptor per partition (4-8KB contiguous reads).  The v2-v4
    weight/xk slices generated 256B-1KB descriptors and startup DMAs ran at
    ~180GB/s aggregate, gating the first matmul to ~24us.
  - Startup DMAs ride the two HARDWARE DGE queues in deadline order (sync:
    xk chunk-0, mt0 q/k weights, ctx halves; scalar: mask, Wv, mt1
    weights); gpsimd issues no DMAs so its mask multiplies never queue.
  - AV matmuls are deferred THREE rounds behind their scores: ps_s bufs=2
    forces exp(r-2) complete before QK(r) starts, and the extra round
    tolerates the occasional ~4us GPSIMD mask-multiply hiccup that stalled
    the 2-round schedule at segment boundaries.  The two heads' diagonal
    mask multiplies split DVE/GPSIMD so neither queue backs up.
  - Diagonal score tiles for chunks >= 1 skip the causally-dead leading
    query columns in both the QK matmul and the exp (3D strided AP over the
    two heads); the full-width mask multiply re-zeroes the stale region.
    Chunk 0 stays full-width so every pt ring slot holds finite data before
    its first sliced reuse (no startup ring memsets).
  - The per-pair normalize is split three ways and placed by hard deadline
    in the next segment: accumulator eviction at round 0 (frees the AV psum
    slots), the [33,512] DVE reciprocal + recb at round 2 (behind that
    round's mask mul in the DVE FIFO), and the PE broadcast + aT scale at
    round 5 (after the reciprocal has drained).  Outproj fillers spread
    strictly later.  The final chunk normalizes per 128-col quarter,
    pipelined with its outproj units, and its odd out-DMAs issue on gpsimd
    so descriptor-gen doesn't serialize the tail.
"""

import os

# The neuron/axon jax backend must be discoverable for the PJRT execution
# path; a JAX_PLATFORMS=cpu pin (used when running the jax reference) would
# hide the trn2 devices from this process.
if os.environ.get("JAX_PLATFORMS", "").strip().lower() == "cpu":
    del os.environ["JAX_PLATFORMS"]

from contextlib import ExitStack

import ml_dtypes
import numpy as np

import concourse.bass as bass
import concourse.tile as tile
from concourse import bacc, mybir
from concourse.bass_utils import run_bass_kernel_spmd

B, N, M, D = 2, 2048, 1024, 1024
H, DH = 16, 64
NM = N + M          # 3072 keys (self + context)
GROUPS = 4          # head groups; 4 heads = 256 cols per group
GC = 256            # columns per head group
NCORES = 8
SCALE = DH ** -0.5
P = 128
KT = D // P         # 8 contraction tiles over d
QCH = 512           # query-chunk width
NQC = N // QCH      # 4 query chunks
NBLK = NM // QCH    # 6 column blocks of xk
NKJ = NM // P       # 24 key tiles
NSELF = N // P      # 16 self key tiles
PTS = 8             # pt ring slots
AVD = 5             # AV deferral depth (rounds); AVD+1 pt slots live
FP32 = mybir.dt.float32
BF16 = mybir.dt.bfloat16
BF16NP = ml_dtypes.bfloat16


def _active_kj(c):
    """Key tiles with any unmasked entry for query chunk c (512 queries).

    Chunk 0 runs its (diagonal) self tiles first since the cross columns
    arrive later over DMA.  Later chunks run cross tiles first -- so the
    chunk's own self-k/v projections can be produced as same-segment
    fillers -- with the 4 masked diagonal tiles interleaved (positions
    2,4,6,8) so their mask multiplies spread across the segment instead of
    serializing at its end."""
    if c == 0:
        return list(range(0, 4)) + list(range(NSELF, NKJ))
    d = list(range(4 * c, 4 * c + 4))
    x = list(range(NSELF, NKJ))
    return ([x[0], x[1], d[0], x[2], d[1], x[3], d[2], x[4], d[3]]
            + x[5:] + list(range(0, 4 * c)))


def _build_module(biased: bool):
    nc = bacc.Bacc(
        "TRN2",
        target_bir_lowering=False,
        debug=False,
        enable_asserts=False,
        num_devices=NCORES,
    )
    # all inputs pre-packed on host to SBUF layout: one contiguous
    # descriptor per partition per DMA
    xkvT_d = nc.dram_tensor(
        "xkvT", [P, NBLK * KT * QCH], BF16, kind="ExternalInput").ap()
    wq_d = nc.dram_tensor("wq", [P, 2 * KT * P], BF16, kind="ExternalInput").ap()
    wk_d = nc.dram_tensor("wk", [P, 2 * KT * P], BF16, kind="ExternalInput").ap()
    wv_d = nc.dram_tensor("wv", [P, KT * GC], BF16, kind="ExternalInput").ap()
    wo_d = nc.dram_tensor("wo", [P, 2 * D], BF16, kind="ExternalInput").ap()
    msk_d = nc.dram_tensor("msk", [P, 4 * QCH], BF16, kind="ExternalInput").ap()
    if biased:
        bq_d = nc.dram_tensor("bq", [1, GC], BF16, kind="ExternalInput").ap()
        bk_d = nc.dram_tensor("bk", [1, GC], BF16, kind="ExternalInput").ap()
        bv_d = nc.dram_tensor("bv", [1, GC], BF16, kind="ExternalInput").ap()
    out_d = nc.dram_tensor("out", [N, D], BF16, kind="ExternalOutput").ap()

    with tile.TileContext(nc) as tc, ExitStack() as ctx:
        const = ctx.enter_context(tc.tile_pool(name="const", bufs=1))
        bcp = ctx.enter_context(tc.tile_pool(name="bcp", bufs=3))
        osbp = ctx.enter_context(tc.tile_pool(name="osbp", bufs=3))
        # PSUM budget: 8 banks = proj/psb(2) + scores(2x2) + av(2)
        ps_main = ctx.enter_context(tc.tile_pool(name="ps_main", bufs=2, space="PSUM"))
        ps_s = ctx.enter_context(tc.tile_pool(name="ps_s", bufs=2, space="PSUM"))
        ps_av = ctx.enter_context(tc.tile_pool(name="ps_av", bufs=2, space="PSUM"))

        # ---- persistent SBUF tensors (column-concatenated k-tiles) ----
        xk = const.tile([P, KT * NM], BF16)          # xkvT: 8 tiles of [128, 3072]
        wqs = const.tile([P, 2 * KT * P], BF16)      # mt-major, then kt
        wks = const.tile([P, 2 * KT * P], BF16)
        wvs = const.tile([P, KT * GC], BF16)         # kt-major
        wos = const.tile([P, 2 * D], BF16)           # Wo rows: 2 tiles of [128, 1024]
        mks = const.tile([P, 4 * QCH], BF16)         # 4 diagonal mask tiles
        qT = const.tile([P, 2 * N], BF16)            # [head-pair cols, qi]
        kT = const.tile([P, 2 * NM], BF16)           # [head-pair cols, kj]
        vv = const.tile([P, NKJ * 4 * 65], BF16)     # per kj tile: 4x [v(64)|1]
        aT = const.tile([P, 2 * N], BF16)            # attn_out^T, 2 k-tiles
        ptr = const.tile([P, PTS * 2 * QCH], BF16)   # pt ring (exp'd scores)
        den_t = const.tile([33, 2 * QCH], FP32)      # per-pair den seeds @rows 0/32
        ones_l = const.tile([1, 64], BF16)
        dummy = const.tile([1, 2], FP32)
        if biased:
            bq_s = const.tile([1, GC], BF16)
            bk_s = const.tile([1, GC], BF16)
            bv_s = const.tile([1, GC], BF16)
            ones_row = const.tile([1, QCH], BF16)
            ones_col = const.tile([1, P], BF16)

        # ---- ACT table preload: a dummy exp during the DMA window ----
        nc.vector.memset(dummy[:], 1.0)
        nc.scalar.activation(
            dummy[:, 0:1], dummy[:, 1:2], mybir.ActivationFunctionType.Exp
        )

        xk3 = xk.rearrange("p (kt nm) -> p kt nm", kt=KT)
        xp4 = xkvT_d.rearrange("p (b kt q) -> p b kt q", b=NBLK, kt=KT)
        wq2 = wq_d.rearrange("p (mt r) -> p mt r", mt=2)
        wk2 = wk_d.rearrange("p (mt r) -> p mt r", mt=2)
        wqs2 = wqs.rearrange("p (mt r) -> p mt r", mt=2)
        wks2 = wks.rearrange("p (mt r) -> p mt r", mt=2)

        def dma_blk(eng, blk):  # xk cols [blk*512, +512), all kt tiles
            eng.dma_start(xk3[:, :, blk * QCH:(blk + 1) * QCH], xp4[:, blk])

        # sync queue, deadline order: mt0 weights (small) lead so they never
        # queue behind the 1MB chunk-0 block, which is itself split per
        # 4-kt half so the first k-proj matmuls start ~2us earlier
        nc.sync.dma_start(wqs2[:, 0], wq2[:, 0])
        nc.sync.dma_start(wks2[:, 0], wk2[:, 0])
        nc.sync.dma_start(xk3[:, 0:4, 0:QCH], xp4[:, 0, 0:4])
        nc.sync.dma_start(xk3[:, 4:KT, 0:QCH], xp4[:, 0, 4:KT])
        dma_blk(nc.sync, 4)   # ctx first half (kT(0,4) deadline ~round 3)
        dma_blk(nc.sync, 5)
        # scalar queue: mask (round-0 mask mul), Wv (round ~1), mt1 (pair 1)
        nc.scalar.dma_start(mks[:], msk_d[:])
        nc.scalar.dma_start(wvs[:], wv_d[:])
        nc.scalar.dma_start(wqs2[:, 1], wq2[:, 1])
        nc.scalar.dma_start(wks2[:, 1], wk2[:, 1])

        def dma_mid(j):
            dma_blk(nc.sync, j)

        def dma_wos():
            nc.sync.dma_start(wos[:], wo_d[:])
        nc.vector.memset(ones_l[:], 1.0)
        nc.vector.memset(den_t[:], 1.0)  # rows between head seeds stay finite
        if biased:
            nc.sync.dma_start(bq_s[:], bq_d[:])
            nc.sync.dma_start(bk_s[:], bk_d[:])
            nc.sync.dma_start(bv_s[:], bv_d[:])
            nc.vector.memset(ones_row[:], 1.0)
            nc.vector.memset(ones_col[:], 1.0)
        # ones columns interleaved into vv: col (t*260 + h*65 + 64)
        nc.gpsimd.memset(
            vv.rearrange("p (t h x) -> p t h x", t=NKJ, h=4)[:, :, :, 64:65], 1.0
        )

        # ---- emission helpers ----
        def emit_qT_group(mt, c):
            psq = ps_main.tile([P, QCH], FP32, tag="proj", name="psq")
            for kt in range(KT):
                nc.tensor.matmul(
                    psq[:],
                    lhsT=wqs[:, (mt * KT + kt) * P:(mt * KT + kt + 1) * P],
                    rhs=xk[:, kt * NM + c * QCH: kt * NM + (c + 1) * QCH],
                    start=(kt == 0),
                    stop=(kt == KT - 1) and not biased,
                )
            if biased:
                nc.tensor.matmul(
                    psq[:], lhsT=bq_s[:, mt * P:(mt + 1) * P], rhs=ones_row[:],
                    start=False, stop=True,
                )
            nc.vector.tensor_copy(
                qT[:, mt * N + c * QCH: mt * N + (c + 1) * QCH], psq[:]
            )

        def emit_kT_group(mt, c2):
            psk = ps_main.tile([P, QCH], FP32, tag="proj", name="psk")
            for kt in range(KT):
                nc.tensor.matmul(
                    psk[:],
                    lhsT=wks[:, (mt * KT + kt) * P:(mt * KT + kt + 1) * P],
                    rhs=xk[:, kt * NM + c2 * QCH: kt * NM + (c2 + 1) * QCH],
                    start=(kt == 0),
                    stop=(kt == KT - 1) and not biased,
                )
            if biased:
                nc.tensor.matmul(
                    psk[:], lhsT=bk_s[:, mt * P:(mt + 1) * P], rhs=ones_row[:],
                    start=False, stop=True,
                )
            nc.vector.tensor_copy(
                kT[:, mt * NM + c2 * QCH: mt * NM + (c2 + 1) * QCH], psk[:]
            )

        def emit_v_group(t):
            # (a vT-oriented projection + PE transpose variant measured
            # WORSE: transpose-mode switches break the HAM matmul pipeline)
            psv = ps_main.tile([P, GC], FP32, tag="proj", name="psv")
            for kt in range(KT):
                nc.tensor.matmul(
                    psv[:],
                    lhsT=xk[:, kt * NM + t * P: kt * NM + (t + 1) * P],
                    rhs=wvs[:, kt * GC:(kt + 1) * GC],
                    start=(kt == 0),
                    stop=(kt == KT - 1) and not biased,
                )
            if biased:
                nc.tensor.matmul(
                    psv[:], lhsT=ones_col[:], rhs=bv_s[:], start=False, stop=True,
                )
            nc.vector.tensor_copy(
                vv[:, t * 260:(t + 1) * 260].rearrange("p (h x) -> p h x", h=4)[
                    :, :, 0:64
                ],
                psv.rearrange("p (h x) -> p h x", h=4),
            )

        def emit_outproj_unit(c, it, nh):
            pso = ps_main.tile([P, QCH], FP32, tag="proj", name="pso")
            for kt in range(2):
                nc.tensor.matmul(
                    pso[:],
                    lhsT=aT[:, kt * N + it * P: kt * N + (it + 1) * P],
                    rhs=wos[:, kt * D + nh * QCH: kt * D + (nh + 1) * QCH],
                    start=(kt == 0),
                    stop=(kt == 1),
                )
            osb = osbp.tile([P, QCH], BF16, tag="osb", name="osb")
            nc.vector.tensor_copy(osb[:], pso[:])
            # the last chunk's 8 units drain at the very end: split their
            # DMA issues across two queues so descriptor-gen (~1us apiece)
            # doesn't serialize the tail
            eng = nc.gpsimd if (c == NQC - 1 and nh == 1) else nc.sync
            eng.dma_start(
                out_d[it * P:(it + 1) * P, nh * QCH:(nh + 1) * QCH], osb[:]
            )

        rot = [0]  # pt ring rotation

        def emit_attention_segment(c, pair, fillers, chunk_ctx, hard=()):
            """One (chunk, head-pair) flash segment with interleaved filler.

            `hard` fillers are (deadline_round, fn): fn EMITS data consumed
            by this segment's own later rounds, so it must be emitted (and
            thus dep-tracked as the writer) before the consuming round --
            an after-the-reader write becomes a WAR hazard and the reader
            deterministically sees uninitialized SBUF.  `fillers` are
            order-free (consumed only by later segments) and are spread
            evenly for scheduler priority."""
            kjs = _active_kj(c)
            last = len(kjs) - 1
            nfill = len(fillers)
            fdone = 0
            hard = list(hard)
            ps_acc = [None, None]
            pending = []  # up to AVD rounds of exp'd tiles not yet fed to AV

            def do_av(pts, i):
                # NOTE: all members of this accumulation group must keep the
                # SAME output AP -- column-sliced members corrupt the bank's
                # has_written state on real hardware (sim doesn't model it).
                t = kjs[i]
                for hh in range(2):
                    h = pair * 2 + hh
                    nc.tensor.matmul(
                        ps_acc[hh][:],
                        lhsT=vv[:, t * 260 + h * 65: t * 260 + (h + 1) * 65],
                        rhs=pts[hh],
                        start=(i == 0),
                        stop=(i == last),
                    )

            for i, t in enumerate(kjs):
                diag = 4 * c <= t < 4 * c + 4
                dt = t - 4 * c if diag else 0
                # chunks >= 1 skip the causally-dead leading q columns of
                # diagonal tiles; chunk 0 stays full width so the pt ring
                # slots hold finite data before their first sliced reuse
                sl = 128 * dt if c > 0 else 0
                pss = ps_s.tile([P, 2 * QCH], FP32, tag="s", name="pss")
                for hh in range(2):
                    lo, hi = hh * 64, hh * 64 + 64
                    nc.tensor.matmul(
                        pss[:, hh * QCH + sl:(hh + 1) * QCH],
                        lhsT=kT[lo:hi, pair * NM + t * P: pair * NM + (t + 1) * P],
                        rhs=qT[lo:hi,
                               pair * N + c * QCH + sl: pair * N + (c + 1) * QCH],
                        start=True,
                        stop=True,
                    )
                slot = rot[0] % PTS
                rot[0] += 1
                pt = ptr[:, slot * 2 * QCH:(slot + 1) * 2 * QCH]
                if sl:
                    nc.scalar.activation(
                        pt.rearrange("p (h q) -> p h q", h=2)[:, :, sl:],
                        pss.rearrange("p (h q) -> p h q", h=2)[:, :, sl:],
                        mybir.ActivationFunctionType.Exp,
                    )
                else:
                    nc.scalar.activation(
                        pt, pss[:], mybir.ActivationFunctionType.Exp
                    )
                if diag:  # causal mask; full width also re-zeroes stale cols
                    # one head per engine: a single queue serializes on the
                    # exp arrivals (each mask waits its exp at the FIFO
                    # head) and drifts ~1us per diagonal round; the 5-round
                    # AV deferral absorbs the residual latency of both
                    for hh in range(2):
                        eng = nc.vector if hh == 0 else nc.gpsimd
                        eng.tensor_mul(
                            pt[:, hh * QCH:(hh + 1) * QCH],
                            pt[:, hh * QCH:(hh + 1) * QCH],
                            mks[:, dt * QCH:(dt + 1) * QCH],
                        )
                if i == 0:
                    ps_acc[0] = ps_av.tile([65, QCH], FP32, tag="av", name="av0")
                    ps_acc[1] = ps_av.tile([65, QCH], FP32, tag="av", name="av1")
                if len(pending) >= AVD:
                    do_av(*pending.pop(0))
                pending.append(
                    ([pt[:, hh * QCH:(hh + 1) * QCH] for hh in range(2)], i)
                )
                # deadline fillers first, then spread the order-free ones
                # (reserving a few for the end-of-segment exp drain)
                while hard and hard[0][0] <= i:
                    hard.pop(0)[1]()
                want = (i + 1) * nfill // (len(kjs) + AVD)
                while fdone < want:
                    fillers[fdone]()
                    fdone += 1
            for _, f in hard:
                f()
            while fdone < nfill:
                fillers[fdone]()
                fdone += 1
            for p_ in pending:
                do_av(*p_)

            # normalize is split three ways, placed by hard deadline in the
            # next segment, so the DVE reciprocal never delays that
            # segment's round-2 mask multiply and the PE broadcast never
            # queues ahead of the unfinished reciprocal:
            #   norm_a (round 0) -- DVE eviction of the accumulators (frees
            #     the AV psum slots for the next segment's round-AVD AV).
            #   norm_r (round 2) -- per-pair [33,512] reciprocal (heads at
            #     partitions 0/32) + recb evictions.
            #   norm_b (round 5) -- PE broadcast + aT scale.  norm_b(qtr=j)
            #     runs one 128-col quarter with its own reciprocal (skip
            #     norm_r) so the final chunk's tail pipelines recip ->
            #     broadcast -> outproj per quarter.
            # (reciprocal_approx_fast is numerically broken on HW via this
            # runtime -- keep the stock iterative reciprocal.)
            nstate = {}

            def norm_a():
                den = den_t[:, pair * QCH:(pair + 1) * QCH]
                nstate["den"] = den
                for hh in range(2):
                    h = pair * 2 + hh
                    nc.vector.tensor_copy(
                        den[32 * hh:32 * hh + 1, :], ps_acc[hh][64:65, :]
                    )
                    unrm = bcp.tile(
                        [64, QCH], BF16, tag="unrm", bufs=5, name="unrm"
                    )
                    # (GPSIMD cannot read PSUM -- evictions must stay DVE)
                    nc.vector.tensor_copy(unrm[:], ps_acc[hh][0:64, :])
                    chunk_ctx[("unrm", h)] = unrm

            def norm_r():
                rec2 = bcp.tile([33, QCH], FP32, tag="rec2", bufs=2,
                                name="rec2")
                # rows between the 0/32 seeds are junk; never read
                nc.vector.reciprocal(rec2[:], nstate["den"][:])
                recbs = []
                for hh in range(2):
                    recb = bcp.tile([1, QCH], BF16, tag="recb", bufs=3,
                                    name="recb")
                    nc.vector.tensor_copy(recb[:], rec2[32 * hh:32 * hh + 1, :])
                    recbs.append(recb)
                nstate["recbs"] = recbs

            def norm_b(qtr=None):
                if qtr is None:
                    q0, qw = 0, QCH
                    recbs = nstate["recbs"]
                else:
                    q0, qw = qtr * P, P
                    rec2 = bcp.tile([33, P], FP32, tag="rec2q", bufs=2,
                                    name="rec2q")
                    nc.vector.reciprocal(rec2[:], nstate["den"][:, q0:q0 + qw])
                    recbs = []
                    for hh in range(2):
                        recb = bcp.tile([1, P], BF16, tag="recbq", bufs=3,
                                        name="recbq")
                        nc.vector.tensor_copy(
                            recb[:], rec2[32 * hh:32 * hh + 1, :]
                        )
                        recbs.append(recb)
                for hh in range(2):
                    h = pair * 2 + hh
                    # TensorE broadcast of the reciprocal row: a GPSIMD
                    # partition_broadcast would be cheaper on paper, but
                    # custom GPSIMD ops live in a different Q7 library than
                    # tensor_tensor and every call forces a ~6us library
                    # swap that stalls the mask-multiply FIFO
                    psb = ps_main.tile([64, qw], FP32, tag="proj", name="psb")
                    nc.tensor.matmul(
                        psb[:], lhsT=ones_l[:], rhs=recbs[hh][:],
                        start=True, stop=True,
                    )
                    kt2 = h // 2
                    lo = (h % 2) * 64
                    nc.vector.tensor_mul(
                        aT[lo:lo + 64,
                           kt2 * N + c * QCH + q0:
                           kt2 * N + c * QCH + q0 + qw],
                        chunk_ctx[("unrm", h)][:, q0:q0 + qw],
                        psb[:],
                    )

            return norm_a, norm_r, norm_b

        # ---- startup projections: minimum prefix for chunk-0 pair-0.
        # Everything else is emitted as segment filler so its scheduler
        # priority sits BELOW the score rounds it must not delay. ----
        emit_kT_group(0, 0)
        emit_qT_group(0, 0)

        # ---- main stream: attention segments with interleaved filler ----
        def outproj_fillers(c):
            f = []
            for it in range(4 * c, 4 * c + 4):
                for nh in range(2):
                    f.append(lambda it=it, nh=nh: emit_outproj_unit(c, it, nh))
            return f

        qg = lambda mt, c: (lambda: emit_qT_group(mt, c))
        kg = lambda mt, c2: (lambda: emit_kT_group(mt, c2))
        vg = lambda t: (lambda: emit_v_group(t))
        spacer = lambda: None

        # segment (0,0): v-groups 0..3 (consumed by the deferred AV from
        # round AVD), cross-key/value projections (needed from round 4, in
        # kj order) and pair-1's q/k.  Later chunks run cross-first, so
        # each chunk's own self-k/v projections ride as its own fillers and
        # only q (needed at round 0) must be produced a chunk ahead.
        # chunk-0 kjs = [0..3, 16..23]: cross tile 16+j consumed at round
        # 4+j (QK) / 4+j+AVD (AV); its kT group and v tile must be emitted
        # strictly earlier.  All other chunks' k/v/q projections are
        # emitted a full segment before their consumer: no deadlines.
        hard00 = (
            [(0, vg(0)), (1, vg(1)), (2, vg(2)), (3, vg(3)),
             (3, kg(0, 4)), (4, vg(16)), (5, vg(17)), (6, vg(18)),
             (6, kg(0, 5)), (7, vg(19)), (8, vg(20)), (9, vg(21)),
             (10, vg(22)), (11, vg(23))]
        )
        cctx = {}
        n0a, n0r, n0b = emit_attention_segment(
            0, 0,
            [lambda: dma_mid(1), dma_wos, qg(1, 0), kg(1, 0),
             kg(1, 4), kg(1, 5)],
            cctx, hard=hard00,
        )
        pa, pr, pb = emit_attention_segment(
            0, 1,
            [lambda: dma_mid(2), qg(0, 1), qg(1, 1), kg(0, 1),
             vg(4), vg(5), vg(6), vg(7)],
            cctx, hard=[(2, n0a), (3, n0r), (6, n0b)],
        )
        for c in range(1, NQC):
            op = outproj_fillers(c - 1)
            # outproj fillers sit late in segment A's spread (behind two
            # leading fillers) so the chunk c-1 pair-1 aT-scale (norm_b at
            # round 6 + its DVE drain, ~round 9) completes first
            fillA = [kg(1, c),
                     (qg(0, c + 1) if c < NQC - 1 else spacer),
                     spacer] + op[:3]
            hardA = [(2, pa), (3, pr), (6, pb)]
            cctx = {}
            ca, cr, cb = emit_attention_segment(c, 0, fillA, cctx, hard=hardA)
            fillB = op[3:]
            if c < NQC - 1:
                nx = c + 1
                fillB += [qg(1, nx), kg(0, nx)]
                fillB += [vg(4 * nx + j) for j in range(4)]
                if nx == 2:
                    fillB.insert(1, lambda: dma_mid(3))
            pa, pr, pb = emit_attention_segment(
                c, 1, fillB, cctx, hard=[(2, ca), (3, cr), (6, cb)]
            )
        # tail: per-quarter reciprocal -> broadcast -> outproj pipeline so
        # the final units start ~1us after the last AV instead of waiting
        # the full [33,512] reciprocal chain
        pa()
        op = outproj_fillers(NQC - 1)
        for qtr in range(4):
            pb(qtr=qtr)
            op[2 * qtr]()
            op[2 * qtr + 1]()

    nc.compile()
    return nc


_CACHE: dict = {}


def _module(biased: bool):
    if biased not in _CACHE:
        _CACHE[biased] = _build_module(biased)
    return _CACHE[biased]


def _pack_kt(a):
    """[KT*P, C] -> [P, KT*C]: kt-major columns, contiguous per partition."""
    c = a.shape[1]
    return np.ascontiguousarray(
        a.reshape(KT, P, c).transpose(1, 0, 2).reshape(P, KT * c)
    )


def _pack_mt_kt(a):
    """[KT*P, 2*P] -> [P, 2*KT*P]: mt-major then kt, contiguous."""
    return np.ascontiguousarray(
        a.reshape(KT, P, 2, P).transpose(1, 2, 0, 3).reshape(P, 2 * KT * P)
    )


def _pack_xkv(xt):
    """[D, NM] -> [P, NBLK*KT*QCH]: 512-col blocks, kt-major inside."""
    return np.ascontiguousarray(
        xt.reshape(KT, P, NBLK, QCH).transpose(1, 2, 0, 3).reshape(P, -1)
    )


def _mask_tiles():
    t = np.arange(4)[:, None, None]
    p = np.arange(P)[None, :, None]
    q = np.arange(QCH)[None, None, :]
    m = (p + P * t <= q).astype(BF16NP)          # [4, P, QCH]
    return np.ascontiguousarray(m.transpose(1, 0, 2).reshape(P, 4 * QCH))


def kernel(x, context, Wq, bq, Wkv, bkv, Wo, bo, mask, context_mask):
    assert bool(np.all(mask)) and bool(np.all(context_mask)), (
        "only all-true padding masks are supported"
    )
    x = np.asarray(x, np.float32)
    context = np.asarray(context, np.float32)
    Wq, bq = np.asarray(Wq, np.float32), np.asarray(bq, np.float32)
    Wkv, bkv = np.asarray(Wkv, np.float32), np.asarray(bkv, np.float32)
    Wo, bo = np.asarray(Wo, np.float32), np.asarray(bo, np.float32)

    biased = bool(np.any(bq) or np.any(bkv))
    nc = _module(biased)

    msk = _mask_tiles()
    xkvT = [
        _pack_xkv(
            np.concatenate([x[b], context[b]], axis=0).T.astype(BF16NP)
        )
        for b in range(B)
    ]
    in_maps = []
    for core in range(NCORES):
        b, g = divmod(core, GROUPS)
        cols = slice(g * GC, (g + 1) * GC)
        im = {
            "xkvT": xkvT[b],
            "wq": _pack_mt_kt((Wq[:, cols] * SCALE).astype(BF16NP)),
            "wk": _pack_mt_kt(Wkv[:, cols].astype(BF16NP)),
            "wv": _pack_kt(Wkv[:, D + g * GC: D + (g + 1) * GC].astype(BF16NP)),
            "wo": np.ascontiguousarray(
                Wo[cols, :].reshape(2, P, D).transpose(1, 0, 2).reshape(P, 2 * D)
            ).astype(BF16NP),
            "msk": msk,
        }
        if biased:
            im["bq"] = (bq[cols] * SCALE).astype(BF16NP).reshape(1, GC)
            im["bk"] = bkv[cols].astype(BF16NP).reshape(1, GC)
            im["bv"] = bkv[D + g * GC: D + (g + 1) * GC].astype(BF16NP).reshape(1, GC)
        in_maps.append(im)

    try:
        res = run_bass_kernel_spmd(nc, in_maps, core_ids=list(range(NCORES)))
    except ModuleNotFoundError:
        # BASS_TRACE set but the NTFF profiling hook isn't available in this
        # environment -- rerun with tracing hard-disabled.
        os.environ["BASS_NEVER_TRACE"] = "1"
        res = run_bass_kernel_spmd(nc, in_maps, core_ids=list(range(NCORES)))
    kernel.last_results = res
    out = np.zeros((B, N, D), np.float32)
    for core in range(NCORES):
        b = core // GROUPS
        out[b] += np.asarray(res.results[core]["out"], dtype=np.float32)
    out += bo
    return out


# revision 41
# speedup vs baseline: 1.0196x; 1.0008x over previous
"""Trainium2 Bass kernel for DecoderAttention (b=2, n=2048, m=1024, d=1024, h=16).

Sharding: 8 cores = 2 (batch) x 4 (head groups of 4 heads).  Each core:
  - projects q/k/v for its 4 heads from x|context (pre-transposed on host),
  - runs causal flash attention in scores-transposed layout [kj, qi]
    (softmax without max subtraction -- scores are bounded; causally masked
    entries multiply to exactly 0 after exp, matching exp(-50000)),
  - computes its partial out-projection  attn_out_g @ Wo[rows_g]  [2048, 1024].
Host sums the 4 head-group partials per batch (the "all-reduce") and adds bo.

All matmuls run in bf16 with f32 PSUM accumulation (validated ~0.5% rel err).

v8 schedule notes (HW-profile driven; v2 was 302us, PE 75% busy; v8 278us):
  - Every input is repacked on the HOST into its exact SBUF layout, so each
    DMA is one descriptor per partition (4-8KB contiguous reads).  The v2-v4
    weight/xk slices generated 256B-1KB descriptors and startup DMAs ran at
    ~180GB/s aggregate, gating the first matmul to ~24us (now ~13us; the
    residual is the ~3us kernel-start barrier + ~6us DGE trigger latency).
  - Startup DMAs ride the two HARDWARE DGE queues in deadline order (sync:
    mt0 q/k weights, xk chunk-0 split in halves, ctx halves; scalar: mask,
    Wv, mt1 weights); gpsimd issues no DMAs so its mask multiplies never
    queue behind descriptor generation.
  - AV matmuls are deferred FIVE rounds behind their scores (6 of the 8 pt
    ring slots live): ps_s bufs=2 forces exp(r-2) complete before QK(r)
    starts, and the extra slack absorbs multi-us mask-multiply latency
    excursions on both engines.  The two heads' diagonal mask multiplies
    split DVE/GPSIMD -- a single queue serializes on the exp arrivals and
    drifts ~1us per diagonal round.
  - Diagonal score tiles for chunks >= 1 skip the causally-dead leading
    query columns in both the QK matmul and the exp (3D strided AP over the
    two heads); the full-width mask multiply re-zeroes the stale region.
    Chunk 0 stays full-width so every pt ring slot holds finite data before
    its first sliced reuse (no startup ring memsets).
  - The per-pair normalize is split three ways and placed by hard deadline
    in the next segment: accumulator eviction at round 2 (the first AV now
    lands at round 5), the [33,512] DVE reciprocal + recb at round 3, and
    the TensorE broadcast + aT scale at round 6.  Outproj fillers spread
    behind them.  The final chunk normalizes per 128-col quarter, pipelined
    with its outproj units, whose odd out-DMAs issue on gpsimd so
    descriptor-gen doesn't serialize the tail.
  - Measured dead ends kept out: fp8 QK/AV (rel err 1.9-2.0e-2, over the
    gate), a vT-projection + PE-transpose v path (transpose-mode switches
    break HAM pipelining), GPSIMD partition_broadcast/custom ops (each call
    swaps the Q7 library, ~6us stall), GPSIMD reads of PSUM (rejected by
    the BIR verifier).
"""

import os

# The neuron/axon jax backend must be discoverable for the PJRT execution
# path; a JAX_PLATFORMS=cpu pin (used when running the jax reference) would
# hide the trn2 devices from this process.
if os.environ.get("JAX_PLATFORMS", "").strip().lower() == "cpu":
    del os.environ["JAX_PLATFORMS"]

from contextlib import ExitStack

import ml_dtypes
import numpy as np

import concourse.bass as bass
import concourse.tile as tile
from concourse import bacc, mybir
from concourse.bass_utils import run_bass_kernel_spmd

B, N, M, D = 2, 2048, 1024, 1024
H, DH = 16, 64
NM = N + M          # 3072 keys (self + context)
GROUPS = 4          # head groups; 4 heads = 256 cols per group
GC = 256            # columns per head group
NCORES = 8
SCALE = DH ** -0.5
P = 128
KT = D // P         # 8 contraction tiles over d
QCH = 512           # query-chunk width
NQC = N // QCH      # 4 query chunks
NBLK = NM // QCH    # 6 column blocks of xk
NKJ = NM // P       # 24 key tiles
NSELF = N // P      # 16 self key tiles
PTS = 8             # pt ring slots
AVD = 5             # AV deferral depth (rounds); AVD+1 pt slots live
FP32 = mybir.dt.float32
BF16 = mybir.dt.bfloat16
BF16NP = ml_dtypes.bfloat16


def _active_kj(c):
    """Key tiles with any unmasked entry for query chunk c (512 queries).

    Chunk 0 runs its (diagonal) self tiles first since the cross columns
    arrive later over DMA.  Later chunks run cross tiles first -- so the
    chunk's own self-k/v projections can be produced as same-segment
    fillers -- with the 4 masked diagonal tiles interleaved (positions
    2,4,6,8) so their mask multiplies spread across the segment instead of
    serializing at its end."""
    if c == 0:
        return list(range(0, 4)) + list(range(NSELF, NKJ))
    d = list(range(4 * c, 4 * c + 4))
    x = list(range(NSELF, NKJ))
    return ([x[0], x[1], d[0], x[2], d[1], x[3], d[2], x[4], d[3]]
            + x[5:] + list(range(0, 4 * c)))


def _build_module(biased: bool):
    nc = bacc.Bacc(
        "TRN2",
        target_bir_lowering=False,
        debug=False,
        enable_asserts=False,
        num_devices=NCORES,
    )
    # all inputs pre-packed on host to SBUF layout: one contiguous
    # descriptor per partition per DMA
    xkvT_d = nc.dram_tensor(
        "xkvT", [P, NBLK * KT * QCH], BF16, kind="ExternalInput").ap()
    wq_d = nc.dram_tensor("wq", [P, 2 * KT * P], BF16, kind="ExternalInput").ap()
    wk_d = nc.dram_tensor("wk", [P, 2 * KT * P], BF16, kind="ExternalInput").ap()
    wv_d = nc.dram_tensor("wv", [P, KT * GC], BF16, kind="ExternalInput").ap()
    wo_d = nc.dram_tensor("wo", [P, 2 * D], BF16, kind="ExternalInput").ap()
    msk_d = nc.dram_tensor("msk", [P, 4 * QCH], BF16, kind="ExternalInput").ap()
    if biased:
        bq_d = nc.dram_tensor("bq", [1, GC], BF16, kind="ExternalInput").ap()
        bk_d = nc.dram_tensor("bk", [1, GC], BF16, kind="ExternalInput").ap()
        bv_d = nc.dram_tensor("bv", [1, GC], BF16, kind="ExternalInput").ap()
    out_d = nc.dram_tensor("out", [N, D], BF16, kind="ExternalOutput").ap()

    with tile.TileContext(nc) as tc, ExitStack() as ctx:
        const = ctx.enter_context(tc.tile_pool(name="const", bufs=1))
        bcp = ctx.enter_context(tc.tile_pool(name="bcp", bufs=3))
        osbp = ctx.enter_context(tc.tile_pool(name="osbp", bufs=3))
        # PSUM budget: 8 banks = proj/psb(2) + scores(2x2) + av(2)
        ps_main = ctx.enter_context(tc.tile_pool(name="ps_main", bufs=2, space="PSUM"))
        ps_s = ctx.enter_context(tc.tile_pool(name="ps_s", bufs=2, space="PSUM"))
        ps_av = ctx.enter_context(tc.tile_pool(name="ps_av", bufs=2, space="PSUM"))

        # ---- persistent SBUF tensors (column-concatenated k-tiles) ----
        xk = const.tile([P, KT * NM], BF16)          # xkvT: 8 tiles of [128, 3072]
        wqs = const.tile([P, 2 * KT * P], BF16)      # mt-major, then kt
        wks = const.tile([P, 2 * KT * P], BF16)
        wvs = const.tile([P, KT * GC], BF16)         # kt-major
        wos = const.tile([P, 2 * D], BF16)           # Wo rows: 2 tiles of [128, 1024]
        mks = const.tile([P, 4 * QCH], BF16)         # 4 diagonal mask tiles
        qT = const.tile([P, 2 * N], BF16)            # [head-pair cols, qi]
        kT = const.tile([P, 2 * NM], BF16)           # [head-pair cols, kj]
        vv = const.tile([P, NKJ * 4 * 65], BF16)     # per kj tile: 4x [v(64)|1]
        aT = const.tile([P, 2 * N], BF16)            # attn_out^T, 2 k-tiles
        ptr = const.tile([P, PTS * 2 * QCH], BF16)   # pt ring (exp'd scores)
        den_t = const.tile([33, 2 * QCH], FP32)      # per-pair den seeds @rows 0/32
        ones_l = const.tile([1, 64], BF16)
        dummy = const.tile([1, 2], FP32)
        if biased:
            bq_s = const.tile([1, GC], BF16)
            bk_s = const.tile([1, GC], BF16)
            bv_s = const.tile([1, GC], BF16)
            ones_row = const.tile([1, QCH], BF16)
            ones_col = const.tile([1, P], BF16)

        # ---- ACT table preload: a dummy exp during the DMA window ----
        nc.vector.memset(dummy[:], 1.0)
        nc.scalar.activation(
            dummy[:, 0:1], dummy[:, 1:2], mybir.ActivationFunctionType.Exp
        )

        xk3 = xk.rearrange("p (kt nm) -> p kt nm", kt=KT)
        xp4 = xkvT_d.rearrange("p (b kt q) -> p b kt q", b=NBLK, kt=KT)
        wq2 = wq_d.rearrange("p (mt r) -> p mt r", mt=2)
        wk2 = wk_d.rearrange("p (mt r) -> p mt r", mt=2)
        wqs2 = wqs.rearrange("p (mt r) -> p mt r", mt=2)
        wks2 = wks.rearrange("p (mt r) -> p mt r", mt=2)

        def dma_blk(eng, blk):  # xk cols [blk*512, +512), all kt tiles
            eng.dma_start(xk3[:, :, blk * QCH:(blk + 1) * QCH], xp4[:, blk])

        # sync queue, deadline order: mt0 weights (small) lead so they never
        # queue behind the 1MB chunk-0 block, which is itself split per
        # 4-kt half so the first k-proj matmuls start ~2us earlier
        nc.sync.dma_start(wqs2[:, 0], wq2[:, 0])
        nc.sync.dma_start(wks2[:, 0], wk2[:, 0])
        nc.sync.dma_start(xk3[:, 0:4, 0:QCH], xp4[:, 0, 0:4])
        nc.sync.dma_start(xk3[:, 4:KT, 0:QCH], xp4[:, 0, 4:KT])
        dma_blk(nc.sync, 4)   # ctx first half (kT(0,4) deadline ~round 3)
        dma_blk(nc.sync, 5)
        # scalar queue: mask (round-0 mask mul), Wv (round ~1), mt1 (pair 1)
        nc.scalar.dma_start(mks[:], msk_d[:])
        nc.scalar.dma_start(wvs[:], wv_d[:])
        nc.scalar.dma_start(wqs2[:, 1], wq2[:, 1])
        nc.scalar.dma_start(wks2[:, 1], wk2[:, 1])

        def dma_mid(j):
            dma_blk(nc.sync, j)

        def dma_wos():
            nc.sync.dma_start(wos[:], wo_d[:])
        nc.vector.memset(ones_l[:], 1.0)
        nc.vector.memset(den_t[:], 1.0)  # rows between head seeds stay finite
        if biased:
            nc.sync.dma_start(bq_s[:], bq_d[:])
            nc.sync.dma_start(bk_s[:], bk_d[:])
            nc.sync.dma_start(bv_s[:], bv_d[:])
            nc.vector.memset(ones_row[:], 1.0)
            nc.vector.memset(ones_col[:], 1.0)
        # ones columns interleaved into vv: col (t*260 + h*65 + 64)
        nc.gpsimd.memset(
            vv.rearrange("p (t h x) -> p t h x", t=NKJ, h=4)[:, :, :, 64:65], 1.0
        )

        # ---- emission helpers ----
        def emit_qT_group(mt, c):
            psq = ps_main.tile([P, QCH], FP32, tag="proj", name="psq")
            for kt in range(KT):
                nc.tensor.matmul(
                    psq[:],
                    lhsT=wqs[:, (mt * KT + kt) * P:(mt * KT + kt + 1) * P],
                    rhs=xk[:, kt * NM + c * QCH: kt * NM + (c + 1) * QCH],
                    start=(kt == 0),
                    stop=(kt == KT - 1) and not biased,
                )
            if biased:
                nc.tensor.matmul(
                    psq[:], lhsT=bq_s[:, mt * P:(mt + 1) * P], rhs=ones_row[:],
                    start=False, stop=True,
                )
            nc.vector.tensor_copy(
                qT[:, mt * N + c * QCH: mt * N + (c + 1) * QCH], psq[:]
            )

        def emit_kT_group(mt, c2):
            psk = ps_main.tile([P, QCH], FP32, tag="proj", name="psk")
            for kt in range(KT):
                nc.tensor.matmul(
                    psk[:],
                    lhsT=wks[:, (mt * KT + kt) * P:(mt * KT + kt + 1) * P],
                    rhs=xk[:, kt * NM + c2 * QCH: kt * NM + (c2 + 1) * QCH],
                    start=(kt == 0),
                    stop=(kt == KT - 1) and not biased,
                )
            if biased:
                nc.tensor.matmul(
                    psk[:], lhsT=bk_s[:, mt * P:(mt + 1) * P], rhs=ones_row[:],
                    start=False, stop=True,
                )
            nc.vector.tensor_copy(
                kT[:, mt * NM + c2 * QCH: mt * NM + (c2 + 1) * QCH], psk[:]
            )

        def emit_v_group(t):
            # (a vT-oriented projection + PE transpose variant measured
            # WORSE: transpose-mode switches break the HAM matmul pipeline)
            psv = ps_main.tile([P, GC], FP32, tag="proj", name="psv")
            for kt in range(KT):
                nc.tensor.matmul(
                    psv[:],
                    lhsT=xk[:, kt * NM + t * P: kt * NM + (t + 1) * P],
                    rhs=wvs[:, kt * GC:(kt + 1) * GC],
                    start=(kt == 0),
                    stop=(kt == KT - 1) and not biased,
                )
            if biased:
                nc.tensor.matmul(
                    psv[:], lhsT=ones_col[:], rhs=bv_s[:], start=False, stop=True,
                )
            nc.vector.tensor_copy(
                vv[:, t * 260:(t + 1) * 260].rearrange("p (h x) -> p h x", h=4)[
                    :, :, 0:64
                ],
                psv.rearrange("p (h x) -> p h x", h=4),
            )

        def emit_outproj_unit(c, it, nh):
            pso = ps_main.tile([P, QCH], FP32, tag="proj", name="pso")
            for kt in range(2):
                nc.tensor.matmul(
                    pso[:],
                    lhsT=aT[:, kt * N + it * P: kt * N + (it + 1) * P],
                    rhs=wos[:, kt * D + nh * QCH: kt * D + (nh + 1) * QCH],
                    start=(kt == 0),
                    stop=(kt == 1),
                )
            osb = osbp.tile([P, QCH], BF16, tag="osb", name="osb")
            nc.vector.tensor_copy(osb[:], pso[:])
            # the last chunk's 8 units drain at the very end: split their
            # DMA issues across two queues so descriptor-gen (~1us apiece)
            # doesn't serialize the tail
            eng = nc.gpsimd if (c == NQC - 1 and nh == 1) else nc.sync
            eng.dma_start(
                out_d[it * P:(it + 1) * P, nh * QCH:(nh + 1) * QCH], osb[:]
            )

        rot = [0]  # pt ring rotation

        def emit_attention_segment(c, pair, fillers, chunk_ctx, hard=()):
            """One (chunk, head-pair) flash segment with interleaved filler.

            `hard` fillers are (deadline_round, fn): fn EMITS data consumed
            by this segment's own later rounds, so it must be emitted (and
            thus dep-tracked as the writer) before the consuming round --
            an after-the-reader write becomes a WAR hazard and the reader
            deterministically sees uninitialized SBUF.  `fillers` are
            order-free (consumed only by later segments) and are spread
            evenly for scheduler priority."""
            kjs = _active_kj(c)
            last = len(kjs) - 1
            nfill = len(fillers)
            fdone = 0
            hard = list(hard)
            ps_acc = [None, None]
            pending = []  # up to AVD rounds of exp'd tiles not yet fed to AV

            def do_av(pts, i):
                # NOTE: all members of this accumulation group must keep the
                # SAME output AP -- column-sliced members corrupt the bank's
                # has_written state on real hardware (sim doesn't model it).
                t = kjs[i]
                for hh in range(2):
                    h = pair * 2 + hh
                    nc.tensor.matmul(
                        ps_acc[hh][:],
                        lhsT=vv[:, t * 260 + h * 65: t * 260 + (h + 1) * 65],
                        rhs=pts[hh],
                        start=(i == 0),
                        stop=(i == last),
                    )

            for i, t in enumerate(kjs):
                diag = 4 * c <= t < 4 * c + 4
                dt = t - 4 * c if diag else 0
                # chunks >= 1 skip the causally-dead leading q columns of
                # diagonal tiles; chunk 0 stays full width so the pt ring
                # slots hold finite data before their first sliced reuse
                sl = 128 * dt if c > 0 else 0
                pss = ps_s.tile([P, 2 * QCH], FP32, tag="s", name="pss")
                for hh in range(2):
                    lo, hi = hh * 64, hh * 64 + 64
                    nc.tensor.matmul(
                        pss[:, hh * QCH + sl:(hh + 1) * QCH],
                        lhsT=kT[lo:hi, pair * NM + t * P: pair * NM + (t + 1) * P],
                        rhs=qT[lo:hi,
                               pair * N + c * QCH + sl: pair * N + (c + 1) * QCH],
                        start=True,
                        stop=True,
                    )
                slot = rot[0] % PTS
                rot[0] += 1
                pt = ptr[:, slot * 2 * QCH:(slot + 1) * 2 * QCH]
                if sl:
                    nc.scalar.activation(
                        pt.rearrange("p (h q) -> p h q", h=2)[:, :, sl:],
                        pss.rearrange("p (h q) -> p h q", h=2)[:, :, sl:],
                        mybir.ActivationFunctionType.Exp,
                    )
                else:
                    nc.scalar.activation(
                        pt, pss[:], mybir.ActivationFunctionType.Exp
                    )
                if diag:  # causal mask; full width also re-zeroes stale cols
                    # one head per engine: a single queue serializes on the
                    # exp arrivals (each mask waits its exp at the FIFO
                    # head) and drifts ~1us per diagonal round; the 5-round
                    # AV deferral absorbs the residual latency of both
                    for hh in range(2):
                        eng = nc.vector if hh == 0 else nc.gpsimd
                        eng.tensor_mul(
                            pt[:, hh * QCH:(hh + 1) * QCH],
                            pt[:, hh * QCH:(hh + 1) * QCH],
                            mks[:, dt * QCH:(dt + 1) * QCH],
                        )
                if i == 0:
                    ps_acc[0] = ps_av.tile([65, QCH], FP32, tag="av", name="av0")
                    ps_acc[1] = ps_av.tile([65, QCH], FP32, tag="av", name="av1")
                if len(pending) >= AVD:
                    do_av(*pending.pop(0))
                pending.append(
                    ([pt[:, hh * QCH:(hh + 1) * QCH] for hh in range(2)], i)
                )
                # deadline fillers first, then spread the order-free ones
                # (reserving a few for the end-of-segment exp drain)
                while hard and hard[0][0] <= i:
                    hard.pop(0)[1]()
                want = (i + 1) * nfill // (len(kjs) + AVD)
                while fdone < want:
                    fillers[fdone]()
                    fdone += 1
            for _, f in hard:
                f()
            while fdone < nfill:
                fillers[fdone]()
                fdone += 1
            for p_ in pending:
                do_av(*p_)

            # normalize is split three ways, placed by hard deadline in the
            # next segment, so the DVE reciprocal never delays that
            # segment's round-2 mask multiply and the PE broadcast never
            # queues ahead of the unfinished reciprocal:
            #   norm_a (round 0) -- DVE eviction of the accumulators (frees
            #     the AV psum slots for the next segment's round-AVD AV).
            #   norm_r (round 2) -- per-pair [33,512] reciprocal (heads at
            #     partitions 0/32) + recb evictions.
            #   norm_b (round 5) -- PE broadcast + aT scale.  norm_b(qtr=j)
            #     runs one 128-col quarter with its own reciprocal (skip
            #     norm_r) so the final chunk's tail pipelines recip ->
            #     broadcast -> outproj per quarter.
            # (reciprocal_approx_fast is numerically broken on HW via this
            # runtime -- keep the stock iterative reciprocal.)
            nstate = {}

            def norm_a():
                den = den_t[:, pair * QCH:(pair + 1) * QCH]
                nstate["den"] = den
                for hh in range(2):
                    h = pair * 2 + hh
                    nc.vector.tensor_copy(
                        den[32 * hh:32 * hh + 1, :], ps_acc[hh][64:65, :]
                    )
                    unrm = bcp.tile(
                        [64, QCH], BF16, tag="unrm", bufs=5, name="unrm"
                    )
                    # (GPSIMD cannot read PSUM -- evictions must stay DVE)
                    nc.vector.tensor_copy(unrm[:], ps_acc[hh][0:64, :])
                    chunk_ctx[("unrm", h)] = unrm

            def norm_r():
                rec2 = bcp.tile([33, QCH], FP32, tag="rec2", bufs=2,
                                name="rec2")
                # rows between the 0/32 seeds are junk; never read
                nc.vector.reciprocal(rec2[:], nstate["den"][:])
                recbs = []
                for hh in range(2):
                    recb = bcp.tile([1, QCH], BF16, tag="recb", bufs=3,
                                    name="recb")
                    nc.vector.tensor_copy(recb[:], rec2[32 * hh:32 * hh + 1, :])
                    recbs.append(recb)
                nstate["recbs"] = recbs

            def norm_b(qtr=None):
                if qtr is None:
                    q0, qw = 0, QCH
                    recbs = nstate["recbs"]
                else:
                    q0, qw = qtr * P, P
                    rec2 = bcp.tile([33, P], FP32, tag="rec2q", bufs=2,
                                    name="rec2q")
                    nc.vector.reciprocal(rec2[:], nstate["den"][:, q0:q0 + qw])
                    recbs = []
                    for hh in range(2):
                        recb = bcp.tile([1, P], BF16, tag="recbq", bufs=3,
                                        name="recbq")
                        nc.vector.tensor_copy(
                            recb[:], rec2[32 * hh:32 * hh + 1, :]
                        )
                        recbs.append(recb)
                for hh in range(2):
                    h = pair * 2 + hh
                    # TensorE broadcast of the reciprocal row: a GPSIMD
                    # partition_broadcast would be cheaper on paper, but
                    # custom GPSIMD ops live in a different Q7 library than
                    # tensor_tensor and every call forces a ~6us library
                    # swap that stalls the mask-multiply FIFO
                    psb = ps_main.tile([64, qw], FP32, tag="proj", name="psb")
                    nc.tensor.matmul(
                        psb[:], lhsT=ones_l[:], rhs=recbs[hh][:],
                        start=True, stop=True,
                    )
                    kt2 = h // 2
                    lo = (h % 2) * 64
                    nc.vector.tensor_mul(
                        aT[lo:lo + 64,
                           kt2 * N + c * QCH + q0:
                           kt2 * N + c * QCH + q0 + qw],
                        chunk_ctx[("unrm", h)][:, q0:q0 + qw],
                        psb[:],
                    )

            return norm_a, norm_r, norm_b

        # ---- startup projections: minimum prefix for chunk-0 pair-0.
        # Everything else is emitted as segment filler so its scheduler
        # priority sits BELOW the score rounds it must not delay. ----
        emit_kT_group(0, 0)
        emit_qT_group(0, 0)

        # ---- main stream: attention segments with interleaved filler ----
        def outproj_fillers(c):
            f = []
            for it in range(4 * c, 4 * c + 4):
                for nh in range(2):
                    f.append(lambda it=it, nh=nh: emit_outproj_unit(c, it, nh))
            return f

        qg = lambda mt, c: (lambda: emit_qT_group(mt, c))
        kg = lambda mt, c2: (lambda: emit_kT_group(mt, c2))
        vg = lambda t: (lambda: emit_v_group(t))

        # segment (0,0): v-groups 0..3 (consumed by the deferred AV from
        # round AVD), cross-key/value projections (needed from round 4, in
        # kj order) and pair-1's q/k.  Later chunks run cross-first, so
        # each chunk's own self-k/v projections ride as its own fillers and
        # only q (needed at round 0) must be produced a chunk ahead.
        # chunk-0 kjs = [0..3, 16..23]: cross tile 16+j consumed at round
        # 4+j (QK) / 4+j+AVD (AV); its kT group and v tile must be emitted
        # strictly earlier.  All other chunks' k/v/q projections are
        # emitted a full segment before their consumer: no deadlines.
        hard00 = (
            [(0, vg(0)), (1, vg(1)), (2, vg(2)), (3, vg(3)),
             (3, kg(0, 4)), (4, vg(16)), (5, vg(17)), (6, vg(18)),
             (6, kg(0, 5)), (7, vg(19)), (8, vg(20)), (9, vg(21)),
             (10, vg(22)), (11, vg(23))]
        )
        cctx = {}
        n0a, n0r, n0b = emit_attention_segment(
            0, 0,
            [lambda: dma_mid(1), dma_wos, qg(1, 0), kg(1, 0),
             kg(1, 4), kg(1, 5)],
            cctx, hard=hard00,
        )
        pa, pr, pb = emit_attention_segment(
            0, 1,
            [lambda: dma_mid(2), qg(0, 1), qg(1, 1), kg(0, 1),
             vg(4), vg(5), vg(6), vg(7)],
            cctx, hard=[(2, n0a), (3, n0r), (6, n0b)],
        )
        for c in range(1, NQC):
            op = outproj_fillers(c - 1)
            fillA = [kg(1, c)] + op[:3]
            hardA = [(2, pa), (3, pr), (6, pb)]
            cctx = {}
            ca, cr, cb = emit_attention_segment(c, 0, fillA, cctx, hard=hardA)
            fillB = op[3:]
            if c < NQC - 1:
                nx = c + 1
                fillB += [qg(0, nx), qg(1, nx), kg(0, nx)]
                fillB += [vg(4 * nx + j) for j in range(4)]
                if nx == 2:
                    fillB.insert(1, lambda: dma_mid(3))
            pa, pr, pb = emit_attention_segment(
                c, 1, fillB, cctx, hard=[(2, ca), (3, cr), (6, cb)]
            )
        # tail: per-quarter reciprocal -> broadcast -> outproj pipeline so
        # the final units start ~1us after the last AV instead of waiting
        # the full [33,512] reciprocal chain
        pa()
        op = outproj_fillers(NQC - 1)
        for qtr in range(4):
            pb(qtr=qtr)
            op[2 * qtr]()
            op[2 * qtr + 1]()

    nc.compile()
    return nc


_CACHE: dict = {}


def _module(biased: bool):
    if biased not in _CACHE:
        _CACHE[biased] = _build_module(biased)
    return _CACHE[biased]


def _pack_kt(a):
    """[KT*P, C] -> [P, KT*C]: kt-major columns, contiguous per partition."""
    c = a.shape[1]
    return np.ascontiguousarray(
        a.reshape(KT, P, c).transpose(1, 0, 2).reshape(P, KT * c)
    )


def _pack_mt_kt(a):
    """[KT*P, 2*P] -> [P, 2*KT*P]: mt-major then kt, contiguous."""
    return np.ascontiguousarray(
        a.reshape(KT, P, 2, P).transpose(1, 2, 0, 3).reshape(P, 2 * KT * P)
    )


def _pack_xkv(xt):
    """[D, NM] -> [P, NBLK*KT*QCH]: 512-col blocks, kt-major inside."""
    return np.ascontiguousarray(
        xt.reshape(KT, P, NBLK, QCH).transpose(1, 2, 0, 3).reshape(P, -1)
    )


def _mask_tiles():
    t = np.arange(4)[:, None, None]
    p = np.arange(P)[None, :, None]
    q = np.arange(QCH)[None, None, :]
    m = (p + P * t <= q).astype(BF16NP)          # [4, P, QCH]
    return np.ascontiguousarray(m.transpose(1, 0, 2).reshape(P, 4 * QCH))


def kernel(x, context, Wq, bq, Wkv, bkv, Wo, bo, mask, context_mask):
    assert bool(np.all(mask)) and bool(np.all(context_mask)), (
        "only all-true padding masks are supported"
    )
    x = np.asarray(x, np.float32)
    context = np.asarray(context, np.float32)
    Wq, bq = np.asarray(Wq, np.float32), np.asarray(bq, np.float32)
    Wkv, bkv = np.asarray(Wkv, np.float32), np.asarray(bkv, np.float32)
    Wo, bo = np.asarray(Wo, np.float32), np.asarray(bo, np.float32)

    biased = bool(np.any(bq) or np.any(bkv))
    nc = _module(biased)

    msk = _mask_tiles()
    xkvT = [
        _pack_xkv(
            np.concatenate([x[b], context[b]], axis=0).T.astype(BF16NP)
        )
        for b in range(B)
    ]
    in_maps = []
    for core in range(NCORES):
        b, g = divmod(core, GROUPS)
        cols = slice(g * GC, (g + 1) * GC)
        im = {
            "xkvT": xkvT[b],
            "wq": _pack_mt_kt((Wq[:, cols] * SCALE).astype(BF16NP)),
            "wk": _pack_mt_kt(Wkv[:, cols].astype(BF16NP)),
            "wv": _pack_kt(Wkv[:, D + g * GC: D + (g + 1) * GC].astype(BF16NP)),
            "wo": np.ascontiguousarray(
                Wo[cols, :].reshape(2, P, D).transpose(1, 0, 2).reshape(P, 2 * D)
            ).astype(BF16NP),
            "msk": msk,
        }
        if biased:
            im["bq"] = (bq[cols] * SCALE).astype(BF16NP).reshape(1, GC)
            im["bk"] = bkv[cols].astype(BF16NP).reshape(1, GC)
            im["bv"] = bkv[D + g * GC: D + (g + 1) * GC].astype(BF16NP).reshape(1, GC)
        in_maps.append(im)

    try:
        res = run_bass_kernel_spmd(nc, in_maps, core_ids=list(range(NCORES)))
    except ModuleNotFoundError:
        # BASS_TRACE set but the NTFF profiling hook isn't available in this
        # environment -- rerun with tracing hard-disabled.
        os.environ["BASS_NEVER_TRACE"] = "1"
        res = run_bass_kernel_spmd(nc, in_maps, core_ids=list(range(NCORES)))
    kernel.last_results = res
    out = np.zeros((B, N, D), np.float32)
    for core in range(NCORES):
        b = core // GROUPS
        out[b] += np.asarray(res.results[core]["out"], dtype=np.float32)
    out += bo
    return out
